# revision 1
# baseline (speedup 1.0000x reference)
"""CenterLoss forward on 8 Trainium2 NeuronCores.

Reference computation (see problem):
    N = 16*256 = 4096 rows, D = 512, C = 10000 classes
    dist[n] = ||x[n] - centers[labels[n]]||^2
    loss = sum_n clamp(dist[n], 1e-12, 1e12) + N*(C-1)*1e-12
(the constant term comes from the reference clamping the masked-out zero
entries of the full N x C distance matrix to 1e-12 before summing).

Sharding: data-parallel over N. Each of the 8 cores gets 512 rows of x and
labels; centers live (replicated) in each core's DRAM but only the 512
needed rows are read, via indirect (gather) DMAs — 20 MB of centers never
moves. x and centers stream as bf16 (the loss averages ~2M element
contributions, so bf16 input rounding lands ~1e-6..1e-5 relative on the
sum — verified against the f32 reference). Each core returns its 512
per-row squared distances (2 KB, f32); the host clamps and reduces in f64.

Per-core pipeline, rows in RPP=4 chunks of 128 contiguous rows (chunk c,
partition p = shard row c*128 + p):
 1. chunk labels -> [128,1] SBUF tiles (one index per partition, the only
    offset-AP shape the HW DGE gathers correctly), on the sync HWDGE ring;
 2. chunk of x (bf16) -> flat [128, D] tile on the scalar-engine HWDGE
    ring (separate ring, overlaps the label loads);
 3. indirect gather (gpsimd SWDGE) lands the chunk's centers rows (bf16);
 4. DVE: subtract (bf16 2x rate) + fused square-and-row-reduce
    (scalar_tensor_tensor, f32 accum_out), overlapped with later gathers.
"""

import numpy as np

N_CORES = 8
ROWS_TOTAL = 4096
ROWS_PER_CORE = ROWS_TOTAL // N_CORES  # 512
P = 128                                # SBUF partitions
RPP = ROWS_PER_CORE // P               # chunks = rows per partition = 4
D = 512
C = 10000
CLAMP_MIN = 1e-12
CLAMP_MAX = 1e12

_NC_CACHE = {}


def _build_nc():
    import concourse.bacc as bacc
    import concourse.bass as bass
    import concourse.tile as tile
    from concourse import mybir

    nc = bacc.Bacc("TRN2", target_bir_lowering=False)

    f32 = mybir.dt.float32
    bf16 = mybir.dt.bfloat16
    x_d = nc.dram_tensor("x", [ROWS_PER_CORE, D], bf16, kind="ExternalInput")
    lab_d = nc.dram_tensor("labels", [ROWS_PER_CORE], mybir.dt.int32,
                           kind="ExternalInput")
    cen_d = nc.dram_tensor("centers", [C, D], bf16, kind="ExternalInput")
    out_d = nc.dram_tensor("out", [P, RPP], f32, kind="ExternalOutput")

    with tile.TileContext(nc) as tc:
        with tc.tile_pool(name="io", bufs=1) as io, \
             tc.tile_pool(name="work", bufs=2) as work:
            rowsum = io.tile([P, RPP], f32)

            lab_ts, x_ts, g_ts = [], [], []
            for c in range(RPP):
                rows = slice(c * P, (c + 1) * P)
                # 512 B label column: one 4 B descriptor per partition.
                lab_t = io.tile([P, 1], mybir.dt.int32, tag=f"lab{c}")
                nc.sync.dma_start(out=lab_t[:], in_=lab_d[rows, None])
                lab_ts.append(lab_t)

                # x chunk on the ACT HWDGE ring; 128 x 1 KB descriptors.
                x_t = io.tile([P, D], bf16, tag=f"x{c}")
                nc.scalar.dma_start(out=x_t[:], in_=x_d[rows, :])
                x_ts.append(x_t)

                # gather chunk: centers[lab[p]] -> partition p.
                g_t = io.tile([P, D], bf16, tag=f"g{c}")
                nc.gpsimd.indirect_dma_start(
                    out=g_t[:],
                    out_offset=None,
                    in_=cen_d[:, :],
                    in_offset=bass.IndirectOffsetOnAxis(
                        ap=lab_t[:, :1], axis=0),
                )
                g_ts.append(g_t)

            for c in range(RPP):
                d_t = work.tile([P, D], bf16, tag="d")
                nc.vector.tensor_sub(d_t[:], x_ts[c][:], g_ts[c][:])
                sq_t = work.tile([P, D], f32, tag="sq")
                # sq = (d + 0) * d, accum_out = per-row sum(sq); fused on DVE
                # (tensor_tensor_reduce hits an unsupported ISA opcode on
                # this runtime and crashes the exec unit).
                nc.vector.scalar_tensor_tensor(
                    out=sq_t[:],
                    in0=d_t[:],
                    scalar=0.0,
                    in1=d_t[:],
                    op0=mybir.AluOpType.add,
                    op1=mybir.AluOpType.mult,
                    accum_out=rowsum[:, c:c + 1],
                )

            nc.sync.dma_start(out=out_d[:, :], in_=rowsum[:])

    nc.finalize()
    return nc


def _get_nc():
    if "nc" not in _NC_CACHE:
        _NC_CACHE["nc"] = _build_nc()
    return _NC_CACHE["nc"]


def _make_in_maps(x, labels, centers):
    import ml_dtypes
    bf16 = ml_dtypes.bfloat16
    xf = np.ascontiguousarray(np.asarray(x).reshape(ROWS_TOTAL, D)
                              .astype(bf16))
    lab = np.ascontiguousarray(
        np.asarray(labels).reshape(ROWS_TOTAL).astype(np.int32))
    cen = np.ascontiguousarray(np.asarray(centers).astype(bf16))

    in_maps = []
    for k in range(N_CORES):
        sl = slice(k * ROWS_PER_CORE, (k + 1) * ROWS_PER_CORE)
        in_maps.append({"x": xf[sl], "labels": lab[sl], "centers": cen})
    return in_maps


def _collect(results):
    """Device outputs -> full loss (host clamp + reduce)."""
    # out[p, c] = squared distance of shard row c*128 + p -> transpose
    # restores shard row order; cores are concatenated in row order.
    per_row = np.concatenate(
        [r["out"].T.reshape(-1) for r in results]).astype(np.float64)
    total = np.clip(per_row, CLAMP_MIN, CLAMP_MAX).sum()
    total += ROWS_TOTAL * (C - 1) * CLAMP_MIN
    return np.asarray(total, dtype=np.float32)


def kernel(x, labels, centers):
    import time
    from concourse.bass_utils import run_bass_kernel_spmd

    nc = _get_nc()
    in_maps = _make_in_maps(x, labels, centers)
    last_err = None
    for attempt in range(3):
        if attempt:
            time.sleep(30)  # transient device errors recover in <1 min
        try:
            res = run_bass_kernel_spmd(nc, in_maps,
                                       core_ids=list(range(N_CORES)))
            return _collect(res.results)
        except Exception as e:  # noqa: BLE001 - retry any runtime failure
            last_err = e
    raise last_err

